# revision 23
# baseline (speedup 1.0000x reference)
"""GAT forward on 8 Trainium2 NeuronCores — one attention head per core.

Math (per head, all [4096] nodes):
    h   = x @ W                      [N, 128]
    ci  = h @ w_i  (per-node)        [N]
    cj  = h @ w_j  (per-node)        [N]
    e^T[j, i] = exp(leaky_relu(ci[i] + cj[j])) * g[j, i]    (g = 0/1 adjacency)
    yT[f, i] = sum_j h[j, f] * eT[j, i]        (PE matmul, e as moving operand)
    rs[i]    = sum_j eT[j, i]                  (PE matmul vs ones column)
    y[i, f]  = yT[f, i] / rs[i] + (x @ W_r_head)[i, f]     (+ bias on host)

The multiplicative mask is exact: exp(lrelu(s) + (-inf if masked)) == 0 ==
exp(lrelu(s)) * 0.

Layout/scheduling notes:
  - Scores are computed TRANSPOSED (j on partitions) so the adjacency mask
    loads in natural row order and e feeds the PE as the moving operand.
  - Everything on the e datapath is bf16 (DVE 2x/4x perf modes; faster PE
    moving pass). yT/rs accumulate fp32 in PSUM.
  - Two alternative per-j-tile pipelines, mixed across tiles to balance
    ACT vs DVE (both produce identical e):
      ACT path: u = Prelu(ciB + cj_bias); v = Exp(u); e = v * g     (DVE: 1 TT)
      DVE path: exp(lrelu(s)) = max(exp(s), exp(s/5)) and exp(ci+cj) =
                exp(ci)*exp(cj), so with per-half broadcast rows E1i=exp(ci),
                E2i=exp(0.2 ci) and per-node columns E1j=exp(cj), E2j:
                u = E1iB *s E1j; w = E2iB *s E2j; p = max(u, w); e = p * g
                (2 TS + 2 TT on DVE, zero ACT)
    ACT_NUM of the 64 (half, jt) tiles take the ACT path.
  - i is split in two 2048-wide halves. The rowsum PSUM accumulator is a
    single bank [128, 512] with the four 512-col chunks landing on
    partitions 0/32/64/96 (legal matmul output base partitions), so
    yT (4 banks) + rs (1 bank) leave 3 banks for phase 1 — which lets the
    first half's attention loop start right after phase-1 quarters 0,1,
    overlapping quarters 2,3 (ramp kill). Finales are emitted in 512-col
    strips so the last one drains through a short pipelined chain (tail
    kill).
"""
import sys

sys.path.insert(0, "/opt/trn_rl_repo")
from contextlib import ExitStack

import numpy as np
import ml_dtypes

import concourse.bass as bass
import concourse.tile as tile
from concourse import bacc, mybir
from concourse.bass_utils import run_bass_kernel_spmd

dt = mybir.dt
F32, BF16 = dt.float32, dt.bfloat16
AF = mybir.ActivationFunctionType
OP = mybir.AluOpType

N = 4096
IN_F = 512
HF = 128
HEADS = 8
SLOPE = 0.2
HALF = 2048
NJT = N // 128  # 32 j-tiles
NMC = IN_F // 128  # 4 contraction chunks over in-features
NT = 2 * NJT  # 64 (half, jt) tiles total

ACT_NUM = 40  # tiles on the ACT (Prelu+Exp) path; rest take the all-DVE path

_prog = None


def _is_act_tile(t):
    return (t * ACT_NUM) // NT != ((t + 1) * ACT_NUM) // NT


def build_program():
    nc = bacc.Bacc("TRN2", target_bir_lowering=False, debug=False)
    xT_d = nc.dram_tensor("xT", [IN_F, N], BF16, kind="ExternalInput").ap()
    mask_d = nc.dram_tensor("mask", [N, N], BF16, kind="ExternalInput").ap()
    W_d = nc.dram_tensor("W", [IN_F, HF], BF16, kind="ExternalInput").ap()
    Wr_d = nc.dram_tensor("Wr", [IN_F, HF], BF16, kind="ExternalInput").ap()
    wi_d = nc.dram_tensor("wi", [HF, 1], BF16, kind="ExternalInput").ap()
    wj_d = nc.dram_tensor("wj", [HF, 1], BF16, kind="ExternalInput").ap()
    eye_d = nc.dram_tensor("eye", [128, 128], BF16, kind="ExternalInput").ap()
    y_d = nc.dram_tensor("y", [N, HF], F32, kind="ExternalOutput").ap()

    with tile.TileContext(nc) as tc, ExitStack() as ctx:
        persist = ctx.enter_context(tc.tile_pool(name="persist", bufs=1))
        h_sb = persist.tile([128, N], BF16, tag="h")  # h[j,f], slice jt -> j-tile
        resid_sb = persist.tile([128, N], BF16, tag="resid")  # resid[i,f] per i-tile
        ciB = persist.tile([128, N], BF16, tag="ciB")  # ci broadcast along partitions
        E1iB = persist.tile([128, N], BF16, tag="E1iB")  # exp(ci) broadcast
        E2iB = persist.tile([128, N], BF16, tag="E2iB")  # exp(0.2 ci) broadcast
        cjT = persist.tile([128, 2 * NJT], F32, tag="cjT")  # cj[j] cols (even idx)
        E1jT = persist.tile([128, 2 * NJT], F32, tag="E1jT")  # exp(cj)
        E2jT = persist.tile([128, 2 * NJT], F32, tag="E2jT")  # exp(0.2 cj)
        eye_bf = persist.tile([128, 128], BF16, tag="eye_bf")
        eye1_sb = persist.tile([64, 1], F32, tag="eye1")
        ones_b = persist.tile([128, 1], BF16, tag="ones")

        nc.sync.dma_start(eye_bf[:], eye_d)
        nc.vector.memset(eye1_sb[:], 1.0)
        nc.vector.memset(ones_b[:], 1.0)

        # Phase-2 pools opened FIRST: their SBUF is disjoint from phase-1
        # buffers, so attention tiles never wait on projection-buffer releases.
        ph2 = ctx.enter_context(tc.tile_pool(name="ph2", bufs=3))
        inpool = ctx.enter_context(tc.tile_pool(name="inpool", bufs=3))
        epool = ctx.enter_context(tc.tile_pool(name="epool", bufs=3))
        fin = ctx.enter_context(tc.tile_pool(name="fin", bufs=2))
        outp = ctx.enter_context(tc.tile_pool(name="outp", bufs=2))

        QC = 1024
        NQ = N // QC  # 4 quarters

        # ---------- helpers ----------

        def emit_elementwise(i0, jt, half):
            t = half * NJT + jt
            g_t = ph2.tile([128, HALF], BF16, tag="m", bufs=10, name=f"g_{i0}_{jt}")
            nc.gpsimd.dma_start(
                g_t[:], mask_d[jt * 128 : (jt + 1) * 128, i0 : i0 + HALF]
            )
            col = (jt // (NJT // 2)) * NJT + 2 * (jt % (NJT // 2))
            e_t = epool.tile([128, HALF], BF16, tag="e", bufs=6, name=f"e_{i0}_{jt}")
            if _is_act_tile(t):
                u_t = inpool.tile([128, HALF], BF16, tag="u", bufs=4, name=f"u_{i0}_{jt}")
                nc.scalar.activation(
                    u_t[:],
                    ciB[:, i0 : i0 + HALF],
                    AF.Prelu,
                    bias=cjT[:, col : col + 1],
                    alpha=SLOPE,
                )
                v_t = inpool.tile([128, HALF], BF16, tag="v", bufs=4, name=f"v_{i0}_{jt}")
                nc.scalar.activation(v_t[:], u_t[:], AF.Exp)
                nc.vector.tensor_mul(e_t[:], v_t[:], g_t[:])
            else:
                u_t = inpool.tile([128, HALF], BF16, tag="u", bufs=4, name=f"u_{i0}_{jt}")
                nc.vector.tensor_scalar_mul(
                    u_t[:], E1iB[:, i0 : i0 + HALF], E1jT[:, col : col + 1]
                )
                w_t = inpool.tile([128, HALF], BF16, tag="w", bufs=4, name=f"w_{i0}_{jt}")
                nc.vector.tensor_scalar_mul(
                    w_t[:], E2iB[:, i0 : i0 + HALF], E2jT[:, col : col + 1]
                )
                p_t = inpool.tile([128, HALF], BF16, tag="v", bufs=4, name=f"p_{i0}_{jt}")
                nc.vector.tensor_max(p_t[:], u_t[:], w_t[:])
                nc.vector.tensor_mul(e_t[:], p_t[:], g_t[:])
            return e_t

        def emit_jtile(jt, half, yT_ps, rs_ab):
            i0 = half * HALF
            e_t = emit_elementwise(i0, jt, half)
            hr = h_sb[:, jt * 128 : (jt + 1) * 128]
            for c in range(HALF // 512):
                nc.tensor.matmul(
                    yT_ps[:, c * 512 : (c + 1) * 512],
                    hr,
                    e_t[:, c * 512 : (c + 1) * 512],
                    start=(jt == 0),
                    stop=(jt == NJT - 1),
                )
            for c in range(HALF // 512):
                # rowsum chunks land on partitions 0/32 of two PSUM banks.
                # The banks are pre-zeroed by memset, so every matmul
                # accumulates (start=False); two per-bank interleaved
                # start..stop windows would be UB.
                rs_ps = rs_ab[c // 2]
                p0 = 32 * (c % 2)
                nc.tensor.matmul(
                    rs_ps[p0 : p0 + 1, :],
                    ones_b[:],
                    e_t[:, c * 512 : (c + 1) * 512],
                    start=False,
                    stop=(jt == NJT - 1),
                    skip_group_check=True,
                )

        def emit_finale(half, yT_ps, rs_ab, pfin):
            # rowsum -> reciprocal chain (small, emitted first)
            rs_sb = fin.tile([64, 1024], F32, tag="rs_sb")
            nc.vector.tensor_copy(rs_sb[:, 0:512], rs_ab[0][:])
            nc.vector.tensor_copy(rs_sb[:, 512:1024], rs_ab[1][:])
            rsT_ps = pfin.tile([128, HALF // 128], F32, tag="rsT")
            for b in range(HALF // 128):
                c, off = b // 4, (b % 4) * 128
                p0 = 32 * (c % 2)
                nc.tensor.transpose(
                    rsT_ps[:, b : b + 1],
                    rs_sb[p0 : p0 + 1, (c // 2) * 512 + off : (c // 2) * 512 + off + 128],
                    eye1_sb[p0 : p0 + 1, 0:1],
                )
            rsT_sb = fin.tile([128, HALF // 128], F32, tag="rsT_sb")
            nc.vector.tensor_copy(rsT_sb[:], rsT_ps[:])
            recipT = fin.tile([128, HALF // 128], F32, tag="recipT")
            nc.vector.reciprocal(recipT[:], rsT_sb[:])

            # yT strips: copy -> transpose -> combine -> DMA, pipelined
            ytr_sb = fin.tile([128, HALF], BF16, tag="ytr_sb")
            for s in range(HALF // 512):
                yT_sb = fin.tile([128, 512], BF16, tag="yT_sb", bufs=3, name=f"yts_{half}_{s}")
                nc.vector.tensor_copy(yT_sb[:], yT_ps[:, s * 512 : (s + 1) * 512])
                tr_ps = pfin.tile([128, 512], BF16, tag="tr", bufs=1, name=f"tr_{half}_{s}")
                for k in range(4):
                    nc.tensor.transpose(
                        tr_ps[:, k * 128 : (k + 1) * 128],
                        yT_sb[:, k * 128 : (k + 1) * 128],
                        eye_bf[:],
                    )
                nc.vector.tensor_copy(
                    ytr_sb[:, s * 512 : (s + 1) * 512], tr_ps[:]
                )
            return ytr_sb, recipT

        def emit_combine(half, ytr_sb, recipT):
            for gi in range(HALF // 128):
                g = half * (HALF // 128) + gi
                ob = outp.tile([128, HF], F32, tag="ob", bufs=3)
                nc.vector.scalar_tensor_tensor(
                    ob[:],
                    ytr_sb[:, gi * 128 : (gi + 1) * 128],
                    recipT[:, gi : gi + 1],
                    resid_sb[:, g * 128 : (g + 1) * 128],
                    op0=OP.mult,
                    op1=OP.add,
                )
                nc.sync.dma_start(y_d[g * 128 : (g + 1) * 128, :], ob[:])

        # ---------- program ----------

        # pmm0 opens BEFORE the phase-1 pools: tile pools close LIFO, and
        # phase-1 (psb, 2 banks) must release while pmm0 (6 banks) stays live.
        pmm0_ctx = ExitStack()
        pmm0 = pmm0_ctx.enter_context(tc.tile_pool(name="pmm0", bufs=1, space="PSUM"))
        yT0 = pmm0.tile([128, HALF], F32, tag="yT", name="yT0")
        rs0 = (
            pmm0.tile([64, 512], F32, tag="rsa", name="rs0a"),
            pmm0.tile([64, 512], F32, tag="rsb", name="rs0b"),
        )
        nc.vector.memset(rs0[0][:], 0.0)
        nc.vector.memset(rs0[1][:], 0.0)

        with ExitStack() as p1:
            ph1 = p1.enter_context(tc.tile_pool(name="ph1", bufs=1))
            xpool = p1.enter_context(tc.tile_pool(name="xpool", bufs=3))
            psb = p1.enter_context(tc.tile_pool(name="psb", bufs=1, space="PSUM"))

            W_sb = ph1.tile([128, NMC * HF], BF16, tag="W")
            Wr_sb = ph1.tile([128, NMC * HF], BF16, tag="Wr")
            # one strided DMA each: dst col block mc <- DRAM rows mc*128..
            nc.sync.dma_start(
                W_sb[:], W_d.rearrange("(m p) f -> p m f", m=NMC)
            )
            nc.sync.dma_start(
                Wr_sb[:], Wr_d.rearrange("(m p) f -> p m f", m=NMC)
            )
            wi_sb = ph1.tile([128, 1], BF16, tag="wi")
            nc.sync.dma_start(wi_sb[:], wi_d)
            wj_sb = ph1.tile([128, 1], BF16, tag="wj")
            nc.sync.dma_start(wj_sb[:], wj_d)
            # wj padded to 2 columns so the moving free dim stays 4B-aligned
            wj2 = ph1.tile([128, 2], BF16, tag="wj2")
            nc.vector.memset(wj2[:], 0.0)
            nc.vector.tensor_copy(wj2[:, 0:1], wj_sb[:])

            hT_sb = ph1.tile([128, N], BF16, tag="hT")  # hT[f, j]
            ci_rowh = ph1.tile([1, N], BF16, tag="ci_row")

            # phase-1 PSUM: psA 1 bank + psC 1 bank = 2 banks, so it coexists
            # with the half-0 attention accumulators (4 yT + 2 rs banks).
            # hT accumulates in psA while resid accumulates in psC (separate
            # windows, pipelined per-mc); ci reuses psA after the hT evac.
            xq_tiles = {}

            def p_xload(q):
                # quarter-wide x loads: 4 DMAs of [128, 1024] per quarter
                o = q * QC
                xts = []
                for mc in range(NMC):
                    xt = xpool.tile(
                        [128, QC], BF16, tag="xt", bufs=6, name=f"xt_{q}_{mc}"
                    )
                    nc.sync.dma_start(
                        xt[:], xT_d[mc * 128 : (mc + 1) * 128, o : o + QC]
                    )
                    xts.append(xt)
                xq_tiles[q] = xts

            def p_subchunk(q, sc):
                oo = q * QC + sc * 512
                ps_hT = psb.tile([128, 512], F32, tag="psA", name=f"phT_{q}_{sc}")
                ps_res = psb.tile([128, 512], F32, tag="psC", name=f"pres_{q}_{sc}")
                for mc in range(NMC):
                    xt = xq_tiles[q][mc][:, sc * 512 : (sc + 1) * 512]
                    nc.tensor.matmul(
                        ps_hT[:],
                        W_sb[:, mc * HF : (mc + 1) * HF],
                        xt,
                        start=(mc == 0),
                        stop=(mc == NMC - 1),
                    )
                    for it in range(4):
                        # exactly ONE start per PSUM bank (first write opens
                        # the accumulation window; overlapping windows are UB)
                        nc.tensor.matmul(
                            ps_res[:, it * 128 : (it + 1) * 128],
                            xt[:, it * 128 : (it + 1) * 128],
                            Wr_sb[:, mc * HF : (mc + 1) * HF],
                            start=(mc == 0 and it == 0),
                            stop=(mc == NMC - 1),
                        )
                nc.vector.tensor_copy(hT_sb[:, oo : oo + 512], ps_hT[:])
                # ci for this subchunk reuses psA once the hT evac clears it
                ps_c = psb.tile([128, 512], F32, tag="psA", name=f"ci_{q}_{sc}")
                nc.tensor.matmul(
                    ps_c[0:1, :],
                    wi_sb[:],
                    hT_sb[:, oo : oo + 512],
                    start=True,
                    stop=True,
                )
                nc.vector.tensor_copy(ci_rowh[0:1, oo : oo + 512], ps_c[0:1, :])
                nc.vector.tensor_copy(resid_sb[:, oo : oo + 512], ps_res[:])

            def p_rows(q):
                o = q * QC
                JQ = QC // 128
                # cj columns for this quarter of j-tiles (8 per quarter).
                # psA is reused here (after the res evac) so cj's accumulation
                # never lands on a bank holding live ci rows.
                ps_c = psb.tile([128, 512], F32, tag="psA", name=f"cj_{q}")
                for k in range(JQ):
                    jt = q * JQ + k
                    nc.tensor.matmul(
                        ps_c[:, 2 * k : 2 * k + 2],
                        hT_sb[:, jt * 128 : (jt + 1) * 128],
                        wj2[:],
                        start=(k == 0),
                        stop=(k == JQ - 1),
                    )
                co = q * 2 * JQ
                nc.vector.tensor_copy(cjT[:, co : co + 2 * JQ], ps_c[:, 0 : 2 * JQ])
                nc.scalar.activation(E1jT[:, co : co + 2 * JQ], ps_c[:, 0 : 2 * JQ], AF.Exp)
                nc.scalar.activation(
                    E2jT[:, co : co + 2 * JQ], ps_c[:, 0 : 2 * JQ], AF.Exp, scale=SLOPE
                )
                nc.gpsimd.partition_broadcast(ciB[:, o : o + QC], ci_rowh[0:1, o : o + QC])
                nc.scalar.activation(E1iB[:, o : o + QC], ciB[:, o : o + QC], AF.Exp)
                nc.scalar.activation(
                    E2iB[:, o : o + QC], ciB[:, o : o + QC], AF.Exp, scale=SLOPE
                )

            def p_htr(q, sc):
                # h[j, f] for 4 j-tiles = transpose(hT) blockwise
                oo = q * QC + sc * 512
                ps_h = psb.tile([128, 512], BF16, tag="psA", name=f"ph_{q}_{sc}")
                for k in range(4):
                    jt = (oo // 128) + k
                    nc.tensor.transpose(
                        ps_h[:, k * 128 : (k + 1) * 128],
                        hT_sb[:, jt * 128 : (jt + 1) * 128],
                        eye_bf[:],
                    )
                nc.vector.tensor_copy(h_sb[:, oo : oo + 512], ps_h[:])

            def emit_quarter(q):
                p_xload(q)
                p_subchunk(q, 0)
                p_subchunk(q, 1)
                p_rows(q)
                p_htr(q, 0)
                p_htr(q, 1)

            # phase-1 quarters 0,1 give half-0 everything it needs for
            # j-tiles 0..15; quarters 2,3 emit piecewise between those tiles
            # so their projections overlap the attention steady state without
            # head-of-line blocking the per-tile e pipeline.
            emit_quarter(0)
            emit_quarter(1)

            for jt in range(8):
                emit_jtile(jt, 0, yT0, rs0)
            q2 = [
                lambda: (p_xload(2), p_subchunk(2, 0)),
                lambda: p_subchunk(2, 1),
                lambda: p_rows(2),
                lambda: p_htr(2, 0),
                lambda: p_htr(2, 1),
            ]
            for jt in range(8, 13):
                emit_jtile(jt, 0, yT0, rs0)
                q2[jt - 8]()
            for jt in range(13, 16):
                emit_jtile(jt, 0, yT0, rs0)
            q3 = [
                lambda: (p_xload(3), p_subchunk(3, 0)),
                lambda: p_subchunk(3, 1),
                lambda: p_rows(3),
                lambda: p_htr(3, 0),
                lambda: p_htr(3, 1),
            ]
            for jt in range(16, 21):
                emit_jtile(jt, 0, yT0, rs0)
                q3[jt - 16]()
            for jt in range(21, NJT):
                emit_jtile(jt, 0, yT0, rs0)

        # half-0 finale + half-1 loop
        with ExitStack() as pf_ctx:
            pfin0 = pf_ctx.enter_context(
                tc.tile_pool(name="pfin0", bufs=1, space="PSUM")
            )
            ytr0, recip0 = emit_finale(0, yT0, rs0, pfin0)
        pmm0_ctx.close()
        emit_combine(0, ytr0, recip0)

        with ExitStack() as pmm1_ctx:
            pmm1 = pmm1_ctx.enter_context(
                tc.tile_pool(name="pmm1", bufs=1, space="PSUM")
            )
            yT1 = pmm1.tile([128, HALF], F32, tag="yT")
            rs1 = (
                pmm1.tile([64, 512], F32, tag="rsa", name="rs1a"),
                pmm1.tile([64, 512], F32, tag="rsb", name="rs1b"),
            )
            nc.vector.memset(rs1[0][:], 0.0)
            nc.vector.memset(rs1[1][:], 0.0)
            for jt in range(NJT):
                emit_jtile(jt, 1, yT1, rs1)

            with ExitStack() as pf_ctx:
                pfin1 = pf_ctx.enter_context(
                    tc.tile_pool(name="pfin1", bufs=1, space="PSUM")
                )
                ytr1, recip1 = emit_finale(1, yT1, rs1, pfin1)
        emit_combine(1, ytr1, recip1)

    nc.compile()
    return nc


def _get_program():
    global _prog
    if _prog is None:
        _prog = build_program()
    return _prog


def _prepare_in_maps(x, graph, W, w_i, w_j, W_r):
    bf = ml_dtypes.bfloat16
    xT = np.ascontiguousarray(x.T).astype(bf)
    mask = (graph > 0).astype(bf)  # multiplicative 0/1 mask
    eye = np.eye(128, dtype=ml_dtypes.bfloat16)
    in_maps = []
    for c in range(HEADS):
        in_maps.append(
            {
                "xT": xT,
                "mask": mask,
                "W": np.ascontiguousarray(W[c]).astype(bf),
                "Wr": np.ascontiguousarray(W_r[:, c * HF : (c + 1) * HF]).astype(bf),
                "wi": np.ascontiguousarray(w_i[c]).astype(bf),
                "wj": np.ascontiguousarray(w_j[c]).astype(bf),
                "eye": eye,
            }
        )
    return in_maps


def run(inputs, trace=False, **kwargs):
    """Run the SPMD kernel; returns (y_full, BassKernelResults)."""
    x = np.asarray(inputs["x"], dtype=np.float32)
    graph = np.asarray(inputs["graph"])
    W = np.asarray(inputs["W"], dtype=np.float32)
    w_i = np.asarray(inputs["w_i"], dtype=np.float32)
    w_j = np.asarray(inputs["w_j"], dtype=np.float32)
    W_r = np.asarray(inputs["W_r"], dtype=np.float32)
    bias = np.asarray(inputs["bias"], dtype=np.float32)

    nc = _get_program()
    in_maps = _prepare_in_maps(x, graph, W, w_i, w_j, W_r)
    br = run_bass_kernel_spmd(
        nc, in_maps, core_ids=list(range(HEADS)), trace=trace, **kwargs
    )
    y = np.concatenate([br.results[c]["y"] for c in range(HEADS)], axis=1)
    y = y + bias[None, :]
    return y.astype(np.float32), br


def kernel(**inputs):
    y, _ = run(inputs)
    return y


# revision 24
# speedup vs baseline: 1.0671x; 1.0671x over previous
"""GAT forward on 8 Trainium2 NeuronCores — one attention head per core.

Math (per head, all [4096] nodes):
    h   = x @ W                      [N, 128]
    ci  = h @ w_i  (per-node)        [N]
    cj  = h @ w_j  (per-node)        [N]
    e^T[j, i] = exp(leaky_relu(ci[i] + cj[j])) * g[j, i]    (g = 0/1 adjacency)
    yT[f, i] = sum_j h[j, f] * eT[j, i]        (PE matmul, e as moving operand)
    rs[i]    = sum_j eT[j, i]                  (PE matmul vs ones column)
    y[i, f]  = yT[f, i] / rs[i] + (x @ W_r_head)[i, f]     (+ bias on host)

The multiplicative mask is exact: exp(lrelu(s) + (-inf if masked)) == 0 ==
exp(lrelu(s)) * 0.

Layout/scheduling notes:
  - Scores are computed TRANSPOSED (j on partitions) so the adjacency mask
    loads in natural row order and e feeds the PE as the moving operand.
  - Everything on the e datapath is bf16 (DVE 2x/4x perf modes; faster PE
    moving pass). yT/rs accumulate fp32 in PSUM.
  - Two alternative per-j-tile pipelines, mixed across tiles to balance
    ACT vs DVE (both produce identical e):
      ACT path: u = Prelu(ciB + cj_bias); v = Exp(u); e = v * g     (DVE: 1 TT)
      DVE path: exp(lrelu(s)) = max(exp(s), exp(s/5)) and exp(ci+cj) =
                exp(ci)*exp(cj), so with per-half broadcast rows E1i=exp(ci),
                E2i=exp(0.2 ci) and per-node columns E1j=exp(cj), E2j:
                u = E1iB *s E1j; w = E2iB *s E2j; p = max(u, w); e = p * g
                (2 TS + 2 TT on DVE, zero ACT)
    ACT_NUM of the 64 (half, jt) tiles take the ACT path.
  - i is split in two 2048-wide halves. The rowsum PSUM accumulator is a
    single bank [128, 512] with the four 512-col chunks landing on
    partitions 0/32/64/96 (legal matmul output base partitions), so
    yT (4 banks) + rs (1 bank) leave 3 banks for phase 1 — which lets the
    first half's attention loop start right after phase-1 quarters 0,1,
    overlapping quarters 2,3 (ramp kill). Finales are emitted in 512-col
    strips so the last one drains through a short pipelined chain (tail
    kill).
"""
import sys

sys.path.insert(0, "/opt/trn_rl_repo")
from contextlib import ExitStack

import numpy as np
import ml_dtypes

import concourse.bass as bass
import concourse.tile as tile
from concourse import bacc, mybir
from concourse.bass_utils import run_bass_kernel_spmd

dt = mybir.dt
F32, BF16 = dt.float32, dt.bfloat16
AF = mybir.ActivationFunctionType
OP = mybir.AluOpType

N = 4096
IN_F = 512
HF = 128
HEADS = 8
SLOPE = 0.2
HALF = 2048
NJT = N // 128  # 32 j-tiles
NMC = IN_F // 128  # 4 contraction chunks over in-features
NT = 2 * NJT  # 64 (half, jt) tiles total

ACT_NUM = 40  # tiles on the ACT (Prelu+Exp) path; rest take the all-DVE path

_prog = None


def _is_act_tile(t):
    return (t * ACT_NUM) // NT != ((t + 1) * ACT_NUM) // NT


def build_program():
    nc = bacc.Bacc("TRN2", target_bir_lowering=False, debug=False)
    xT_d = nc.dram_tensor("xT", [IN_F, N], BF16, kind="ExternalInput").ap()
    mask_d = nc.dram_tensor("mask", [N, N], BF16, kind="ExternalInput").ap()
    W_d = nc.dram_tensor("W", [IN_F, HF], BF16, kind="ExternalInput").ap()
    Wr_d = nc.dram_tensor("Wr", [IN_F, HF], BF16, kind="ExternalInput").ap()
    wi_d = nc.dram_tensor("wi", [HF, 1], BF16, kind="ExternalInput").ap()
    wj_d = nc.dram_tensor("wj", [HF, 1], BF16, kind="ExternalInput").ap()
    eye_d = nc.dram_tensor("eye", [128, 128], BF16, kind="ExternalInput").ap()
    y_d = nc.dram_tensor("y", [N, HF], F32, kind="ExternalOutput").ap()

    with tile.TileContext(nc) as tc, ExitStack() as ctx:
        persist = ctx.enter_context(tc.tile_pool(name="persist", bufs=1))
        h_sb = persist.tile([128, N], BF16, tag="h")  # h[j,f], slice jt -> j-tile
        resid_sb = persist.tile([128, N], BF16, tag="resid")  # resid[i,f] per i-tile
        ciB = persist.tile([128, N], BF16, tag="ciB")  # ci broadcast along partitions
        E1iB = persist.tile([128, N], BF16, tag="E1iB")  # exp(ci) broadcast
        E2iB = persist.tile([128, N], BF16, tag="E2iB")  # exp(0.2 ci) broadcast
        cjT = persist.tile([128, 2 * NJT], F32, tag="cjT")  # cj[j] cols (even idx)
        E1jT = persist.tile([128, 2 * NJT], F32, tag="E1jT")  # exp(cj)
        E2jT = persist.tile([128, 2 * NJT], F32, tag="E2jT")  # exp(0.2 cj)
        eye_bf = persist.tile([128, 128], BF16, tag="eye_bf")
        eye1_sb = persist.tile([64, 1], F32, tag="eye1")
        ones_b = persist.tile([128, 1], BF16, tag="ones")

        nc.sync.dma_start(eye_bf[:], eye_d)
        nc.vector.memset(eye1_sb[:], 1.0)
        nc.vector.memset(ones_b[:], 1.0)

        # Phase-2 pools opened FIRST: their SBUF is disjoint from phase-1
        # buffers, so attention tiles never wait on projection-buffer releases.
        ph2 = ctx.enter_context(tc.tile_pool(name="ph2", bufs=3))
        inpool = ctx.enter_context(tc.tile_pool(name="inpool", bufs=3))
        epool = ctx.enter_context(tc.tile_pool(name="epool", bufs=3))
        fin = ctx.enter_context(tc.tile_pool(name="fin", bufs=2))
        outp = ctx.enter_context(tc.tile_pool(name="outp", bufs=2))

        QC = 1024
        NQ = N // QC  # 4 quarters

        # ---------- helpers ----------

        def emit_elementwise(i0, jt, half):
            t = half * NJT + jt
            g_t = ph2.tile([128, HALF], BF16, tag="m", bufs=10, name=f"g_{i0}_{jt}")
            nc.sync.dma_start(
                g_t[:], mask_d[jt * 128 : (jt + 1) * 128, i0 : i0 + HALF]
            )
            col = (jt // (NJT // 2)) * NJT + 2 * (jt % (NJT // 2))
            e_t = epool.tile([128, HALF], BF16, tag="e", bufs=6, name=f"e_{i0}_{jt}")
            if _is_act_tile(t):
                u_t = inpool.tile([128, HALF], BF16, tag="u", bufs=4, name=f"u_{i0}_{jt}")
                nc.scalar.activation(
                    u_t[:],
                    ciB[:, i0 : i0 + HALF],
                    AF.Prelu,
                    bias=cjT[:, col : col + 1],
                    alpha=SLOPE,
                )
                v_t = inpool.tile([128, HALF], BF16, tag="v", bufs=4, name=f"v_{i0}_{jt}")
                nc.scalar.activation(v_t[:], u_t[:], AF.Exp)
                nc.vector.tensor_mul(e_t[:], v_t[:], g_t[:])
            else:
                u_t = inpool.tile([128, HALF], BF16, tag="u", bufs=4, name=f"u_{i0}_{jt}")
                nc.vector.tensor_scalar_mul(
                    u_t[:], E1iB[:, i0 : i0 + HALF], E1jT[:, col : col + 1]
                )
                w_t = inpool.tile([128, HALF], BF16, tag="w", bufs=4, name=f"w_{i0}_{jt}")
                nc.vector.tensor_scalar_mul(
                    w_t[:], E2iB[:, i0 : i0 + HALF], E2jT[:, col : col + 1]
                )
                p_t = inpool.tile([128, HALF], BF16, tag="v", bufs=4, name=f"p_{i0}_{jt}")
                nc.vector.tensor_max(p_t[:], u_t[:], w_t[:])
                nc.vector.tensor_mul(e_t[:], p_t[:], g_t[:])
            return e_t

        def emit_jtile(jt, half, yT_ps, rs_ab):
            i0 = half * HALF
            e_t = emit_elementwise(i0, jt, half)
            hr = h_sb[:, jt * 128 : (jt + 1) * 128]
            for c in range(HALF // 512):
                nc.tensor.matmul(
                    yT_ps[:, c * 512 : (c + 1) * 512],
                    hr,
                    e_t[:, c * 512 : (c + 1) * 512],
                    start=(jt == 0),
                    stop=(jt == NJT - 1),
                )
            for c in range(HALF // 512):
                # rowsum chunks land on partitions 0/32 of two PSUM banks.
                # The banks are pre-zeroed by memset, so every matmul
                # accumulates (start=False); two per-bank interleaved
                # start..stop windows would be UB.
                rs_ps = rs_ab[c // 2]
                p0 = 32 * (c % 2)
                nc.tensor.matmul(
                    rs_ps[p0 : p0 + 1, :],
                    ones_b[:],
                    e_t[:, c * 512 : (c + 1) * 512],
                    start=False,
                    stop=(jt == NJT - 1),
                    skip_group_check=True,
                )

        def emit_finale(half, yT_ps, rs_ab, pfin):
            # rowsum -> reciprocal chain (small, emitted first)
            rs_sb = fin.tile([64, 1024], F32, tag="rs_sb")
            nc.vector.tensor_copy(rs_sb[:, 0:512], rs_ab[0][:])
            nc.vector.tensor_copy(rs_sb[:, 512:1024], rs_ab[1][:])
            rsT_ps = pfin.tile([128, HALF // 128], F32, tag="rsT")
            for b in range(HALF // 128):
                c, off = b // 4, (b % 4) * 128
                p0 = 32 * (c % 2)
                nc.tensor.transpose(
                    rsT_ps[:, b : b + 1],
                    rs_sb[p0 : p0 + 1, (c // 2) * 512 + off : (c // 2) * 512 + off + 128],
                    eye1_sb[p0 : p0 + 1, 0:1],
                )
            rsT_sb = fin.tile([128, HALF // 128], F32, tag="rsT_sb")
            nc.vector.tensor_copy(rsT_sb[:], rsT_ps[:])
            recipT = fin.tile([128, HALF // 128], F32, tag="recipT")
            nc.vector.reciprocal(recipT[:], rsT_sb[:])

            # yT strips: copy -> transpose -> combine -> DMA, pipelined
            ytr_sb = fin.tile([128, HALF], BF16, tag="ytr_sb")
            for s in range(HALF // 512):
                yT_sb = fin.tile([128, 512], BF16, tag="yT_sb", bufs=3, name=f"yts_{half}_{s}")
                nc.vector.tensor_copy(yT_sb[:], yT_ps[:, s * 512 : (s + 1) * 512])
                tr_ps = pfin.tile([128, 512], BF16, tag="tr", bufs=1, name=f"tr_{half}_{s}")
                for k in range(4):
                    nc.tensor.transpose(
                        tr_ps[:, k * 128 : (k + 1) * 128],
                        yT_sb[:, k * 128 : (k + 1) * 128],
                        eye_bf[:],
                    )
                nc.vector.tensor_copy(
                    ytr_sb[:, s * 512 : (s + 1) * 512], tr_ps[:]
                )
            return ytr_sb, recipT

        def emit_combine(half, ytr_sb, recipT):
            for gi in range(HALF // 128):
                g = half * (HALF // 128) + gi
                ob = outp.tile([128, HF], F32, tag="ob", bufs=3)
                nc.vector.scalar_tensor_tensor(
                    ob[:],
                    ytr_sb[:, gi * 128 : (gi + 1) * 128],
                    recipT[:, gi : gi + 1],
                    resid_sb[:, g * 128 : (g + 1) * 128],
                    op0=OP.mult,
                    op1=OP.add,
                )
                nc.sync.dma_start(y_d[g * 128 : (g + 1) * 128, :], ob[:])

        # ---------- program ----------

        # pmm0 opens BEFORE the phase-1 pools: tile pools close LIFO, and
        # phase-1 (psb, 2 banks) must release while pmm0 (6 banks) stays live.
        pmm0_ctx = ExitStack()
        pmm0 = pmm0_ctx.enter_context(tc.tile_pool(name="pmm0", bufs=1, space="PSUM"))
        yT0 = pmm0.tile([128, HALF], F32, tag="yT", name="yT0")
        rs0 = (
            pmm0.tile([64, 512], F32, tag="rsa", name="rs0a"),
            pmm0.tile([64, 512], F32, tag="rsb", name="rs0b"),
        )
        nc.vector.memset(rs0[0][:], 0.0)
        nc.vector.memset(rs0[1][:], 0.0)

        with ExitStack() as p1:
            ph1 = p1.enter_context(tc.tile_pool(name="ph1", bufs=1))
            xpool = p1.enter_context(tc.tile_pool(name="xpool", bufs=3))
            psb = p1.enter_context(tc.tile_pool(name="psb", bufs=1, space="PSUM"))

            W_sb = ph1.tile([128, NMC * HF], BF16, tag="W")
            Wr_sb = ph1.tile([128, NMC * HF], BF16, tag="Wr")
            # one strided DMA each: dst col block mc <- DRAM rows mc*128..
            nc.sync.dma_start(
                W_sb[:], W_d.rearrange("(m p) f -> p m f", m=NMC)
            )
            nc.sync.dma_start(
                Wr_sb[:], Wr_d.rearrange("(m p) f -> p m f", m=NMC)
            )
            wi_sb = ph1.tile([128, 1], BF16, tag="wi")
            nc.sync.dma_start(wi_sb[:], wi_d)
            wj_sb = ph1.tile([128, 1], BF16, tag="wj")
            nc.sync.dma_start(wj_sb[:], wj_d)
            # wj padded to 2 columns so the moving free dim stays 4B-aligned
            wj2 = ph1.tile([128, 2], BF16, tag="wj2")
            nc.vector.memset(wj2[:], 0.0)
            nc.vector.tensor_copy(wj2[:, 0:1], wj_sb[:])

            hT_sb = ph1.tile([128, N], BF16, tag="hT")  # hT[f, j]
            ci_rowh = ph1.tile([1, N], BF16, tag="ci_row")

            # phase-1 PSUM: psA 1 bank + psC 1 bank = 2 banks, so it coexists
            # with the half-0 attention accumulators (4 yT + 2 rs banks).
            # hT accumulates in psA while resid accumulates in psC (separate
            # windows, pipelined per-mc); ci reuses psA after the hT evac.
            xq_tiles = {}

            def p_xload(q):
                # quarter-wide x loads: 4 DMAs of [128, 1024] per quarter
                o = q * QC
                xts = []
                for mc in range(NMC):
                    xt = xpool.tile(
                        [128, QC], BF16, tag="xt", bufs=6, name=f"xt_{q}_{mc}"
                    )
                    nc.sync.dma_start(
                        xt[:], xT_d[mc * 128 : (mc + 1) * 128, o : o + QC]
                    )
                    xts.append(xt)
                xq_tiles[q] = xts

            def p_subchunk(q, sc):
                oo = q * QC + sc * 512
                ps_hT = psb.tile([128, 512], F32, tag="psA", name=f"phT_{q}_{sc}")
                ps_res = psb.tile([128, 512], F32, tag="psC", name=f"pres_{q}_{sc}")
                for mc in range(NMC):
                    xt = xq_tiles[q][mc][:, sc * 512 : (sc + 1) * 512]
                    nc.tensor.matmul(
                        ps_hT[:],
                        W_sb[:, mc * HF : (mc + 1) * HF],
                        xt,
                        start=(mc == 0),
                        stop=(mc == NMC - 1),
                    )
                    for it in range(4):
                        # exactly ONE start per PSUM bank (first write opens
                        # the accumulation window; overlapping windows are UB)
                        nc.tensor.matmul(
                            ps_res[:, it * 128 : (it + 1) * 128],
                            xt[:, it * 128 : (it + 1) * 128],
                            Wr_sb[:, mc * HF : (mc + 1) * HF],
                            start=(mc == 0 and it == 0),
                            stop=(mc == NMC - 1),
                        )
                nc.vector.tensor_copy(hT_sb[:, oo : oo + 512], ps_hT[:])
                # ci for this subchunk reuses psA once the hT evac clears it
                ps_c = psb.tile([128, 512], F32, tag="psA", name=f"ci_{q}_{sc}")
                nc.tensor.matmul(
                    ps_c[0:1, :],
                    wi_sb[:],
                    hT_sb[:, oo : oo + 512],
                    start=True,
                    stop=True,
                )
                nc.vector.tensor_copy(ci_rowh[0:1, oo : oo + 512], ps_c[0:1, :])
                nc.vector.tensor_copy(resid_sb[:, oo : oo + 512], ps_res[:])

            def p_rows(q):
                o = q * QC
                JQ = QC // 128
                # cj columns for this quarter of j-tiles (8 per quarter).
                # psA is reused here (after the res evac) so cj's accumulation
                # never lands on a bank holding live ci rows.
                ps_c = psb.tile([128, 512], F32, tag="psA", name=f"cj_{q}")
                for k in range(JQ):
                    jt = q * JQ + k
                    nc.tensor.matmul(
                        ps_c[:, 2 * k : 2 * k + 2],
                        hT_sb[:, jt * 128 : (jt + 1) * 128],
                        wj2[:],
                        start=(k == 0),
                        stop=(k == JQ - 1),
                    )
                co = q * 2 * JQ
                nc.vector.tensor_copy(cjT[:, co : co + 2 * JQ], ps_c[:, 0 : 2 * JQ])
                nc.scalar.activation(E1jT[:, co : co + 2 * JQ], ps_c[:, 0 : 2 * JQ], AF.Exp)
                nc.scalar.activation(
                    E2jT[:, co : co + 2 * JQ], ps_c[:, 0 : 2 * JQ], AF.Exp, scale=SLOPE
                )
                nc.gpsimd.partition_broadcast(ciB[:, o : o + QC], ci_rowh[0:1, o : o + QC])
                nc.scalar.activation(E1iB[:, o : o + QC], ciB[:, o : o + QC], AF.Exp)
                nc.scalar.activation(
                    E2iB[:, o : o + QC], ciB[:, o : o + QC], AF.Exp, scale=SLOPE
                )

            def p_htr(q, sc):
                # h[j, f] for 4 j-tiles = transpose(hT) blockwise
                oo = q * QC + sc * 512
                ps_h = psb.tile([128, 512], BF16, tag="psA", name=f"ph_{q}_{sc}")
                for k in range(4):
                    jt = (oo // 128) + k
                    nc.tensor.transpose(
                        ps_h[:, k * 128 : (k + 1) * 128],
                        hT_sb[:, jt * 128 : (jt + 1) * 128],
                        eye_bf[:],
                    )
                nc.vector.tensor_copy(h_sb[:, oo : oo + 512], ps_h[:])

            def emit_quarter(q):
                p_xload(q)
                p_subchunk(q, 0)
                p_subchunk(q, 1)
                p_rows(q)
                p_htr(q, 0)
                p_htr(q, 1)

            # phase-1 quarters 0,1 give half-0 everything it needs for
            # j-tiles 0..15; quarters 2,3 emit piecewise between those tiles
            # so their projections overlap the attention steady state without
            # head-of-line blocking the per-tile e pipeline.
            emit_quarter(0)
            emit_quarter(1)

            for jt in range(8):
                emit_jtile(jt, 0, yT0, rs0)
            q2 = [
                lambda: (p_xload(2), p_subchunk(2, 0)),
                lambda: p_subchunk(2, 1),
                lambda: p_rows(2),
                lambda: p_htr(2, 0),
                lambda: p_htr(2, 1),
            ]
            for jt in range(8, 13):
                emit_jtile(jt, 0, yT0, rs0)
                q2[jt - 8]()
            for jt in range(13, 16):
                emit_jtile(jt, 0, yT0, rs0)
            q3 = [
                lambda: (p_xload(3), p_subchunk(3, 0)),
                lambda: p_subchunk(3, 1),
                lambda: p_rows(3),
                lambda: p_htr(3, 0),
                lambda: p_htr(3, 1),
            ]
            for jt in range(16, 21):
                emit_jtile(jt, 0, yT0, rs0)
                q3[jt - 16]()
            for jt in range(21, NJT):
                emit_jtile(jt, 0, yT0, rs0)

        # half-0 finale + half-1 loop
        with ExitStack() as pf_ctx:
            pfin0 = pf_ctx.enter_context(
                tc.tile_pool(name="pfin0", bufs=1, space="PSUM")
            )
            ytr0, recip0 = emit_finale(0, yT0, rs0, pfin0)
        pmm0_ctx.close()
        emit_combine(0, ytr0, recip0)

        with ExitStack() as pmm1_ctx:
            pmm1 = pmm1_ctx.enter_context(
                tc.tile_pool(name="pmm1", bufs=1, space="PSUM")
            )
            yT1 = pmm1.tile([128, HALF], F32, tag="yT")
            rs1 = (
                pmm1.tile([64, 512], F32, tag="rsa", name="rs1a"),
                pmm1.tile([64, 512], F32, tag="rsb", name="rs1b"),
            )
            nc.vector.memset(rs1[0][:], 0.0)
            nc.vector.memset(rs1[1][:], 0.0)
            for jt in range(NJT):
                emit_jtile(jt, 1, yT1, rs1)

            with ExitStack() as pf_ctx:
                pfin1 = pf_ctx.enter_context(
                    tc.tile_pool(name="pfin1", bufs=1, space="PSUM")
                )
                ytr1, recip1 = emit_finale(1, yT1, rs1, pfin1)
        emit_combine(1, ytr1, recip1)

    nc.compile()
    return nc


def _get_program():
    global _prog
    if _prog is None:
        _prog = build_program()
    return _prog


def _prepare_in_maps(x, graph, W, w_i, w_j, W_r):
    bf = ml_dtypes.bfloat16
    xT = np.ascontiguousarray(x.T).astype(bf)
    mask = (graph > 0).astype(bf)  # multiplicative 0/1 mask
    eye = np.eye(128, dtype=ml_dtypes.bfloat16)
    in_maps = []
    for c in range(HEADS):
        in_maps.append(
            {
                "xT": xT,
                "mask": mask,
                "W": np.ascontiguousarray(W[c]).astype(bf),
                "Wr": np.ascontiguousarray(W_r[:, c * HF : (c + 1) * HF]).astype(bf),
                "wi": np.ascontiguousarray(w_i[c]).astype(bf),
                "wj": np.ascontiguousarray(w_j[c]).astype(bf),
                "eye": eye,
            }
        )
    return in_maps


def run(inputs, trace=False, **kwargs):
    """Run the SPMD kernel; returns (y_full, BassKernelResults)."""
    x = np.asarray(inputs["x"], dtype=np.float32)
    graph = np.asarray(inputs["graph"])
    W = np.asarray(inputs["W"], dtype=np.float32)
    w_i = np.asarray(inputs["w_i"], dtype=np.float32)
    w_j = np.asarray(inputs["w_j"], dtype=np.float32)
    W_r = np.asarray(inputs["W_r"], dtype=np.float32)
    bias = np.asarray(inputs["bias"], dtype=np.float32)

    nc = _get_program()
    in_maps = _prepare_in_maps(x, graph, W, w_i, w_j, W_r)
    br = run_bass_kernel_spmd(
        nc, in_maps, core_ids=list(range(HEADS)), trace=trace, **kwargs
    )
    y = np.concatenate([br.results[c]["y"] for c in range(HEADS)], axis=1)
    y = y + bias[None, :]
    return y.astype(np.float32), br


def kernel(**inputs):
    y, _ = run(inputs)
    return y


# revision 25
# speedup vs baseline: 1.0829x; 1.0147x over previous
"""GAT forward on 8 Trainium2 NeuronCores — one attention head per core.

Math (per head, all [4096] nodes):
    h   = x @ W                      [N, 128]
    ci  = h @ w_i  (per-node)        [N]
    cj  = h @ w_j  (per-node)        [N]
    e^T[j, i] = exp(leaky_relu(ci[i] + cj[j])) * g[j, i]    (g = 0/1 adjacency)
    yT[f, i] = sum_j h[j, f] * eT[j, i]        (PE matmul, e as moving operand)
    rs[i]    = sum_j eT[j, i]                  (PE matmul vs ones column)
    y[i, f]  = yT[f, i] / rs[i] + (x @ W_r_head)[i, f]     (+ bias on host)

The multiplicative mask is exact: exp(lrelu(s) + (-inf if masked)) == 0 ==
exp(lrelu(s)) * 0.

Layout/scheduling notes:
  - Scores are computed TRANSPOSED (j on partitions) so the adjacency mask
    loads in natural row order and e feeds the PE as the moving operand.
  - Everything on the e datapath is bf16 (DVE 2x/4x perf modes; faster PE
    moving pass). yT/rs accumulate fp32 in PSUM.
  - Two alternative per-j-tile pipelines, mixed across tiles to balance
    ACT vs DVE (both produce identical e):
      ACT path: u = Prelu(ciB + cj_bias); v = Exp(u); e = v * g     (DVE: 1 TT)
      DVE path: exp(lrelu(s)) = max(exp(s), exp(s/5)) and exp(ci+cj) =
                exp(ci)*exp(cj), so with per-half broadcast rows E1i=exp(ci),
                E2i=exp(0.2 ci) and per-node columns E1j=exp(cj), E2j:
                u = E1iB *s E1j; w = E2iB *s E2j; p = max(u, w); e = p * g
                (2 TS + 2 TT on DVE, zero ACT)
    ACT_NUM of the 64 (half, jt) tiles take the ACT path.
  - i is split in two 2048-wide halves. The rowsum PSUM accumulator is a
    single bank [128, 512] with the four 512-col chunks landing on
    partitions 0/32/64/96 (legal matmul output base partitions), so
    yT (4 banks) + rs (1 bank) leave 3 banks for phase 1 — which lets the
    first half's attention loop start right after phase-1 quarters 0,1,
    overlapping quarters 2,3 (ramp kill). Finales are emitted in 512-col
    strips so the last one drains through a short pipelined chain (tail
    kill).
"""
import sys

sys.path.insert(0, "/opt/trn_rl_repo")
from contextlib import ExitStack

import numpy as np
import ml_dtypes

import concourse.bass as bass
import concourse.tile as tile
from concourse import bacc, mybir
from concourse.bass_utils import run_bass_kernel_spmd

dt = mybir.dt
F32, BF16 = dt.float32, dt.bfloat16
AF = mybir.ActivationFunctionType
OP = mybir.AluOpType

N = 4096
IN_F = 512
HF = 128
HEADS = 8
SLOPE = 0.2
HALF = 2048
NJT = N // 128  # 32 j-tiles
NMC = IN_F // 128  # 4 contraction chunks over in-features
NT = 2 * NJT  # 64 (half, jt) tiles total

ACT_NUM = 40  # tiles on the ACT (Prelu+Exp) path; rest take the all-DVE path

_prog = None


def _is_act_tile(t):
    return (t * ACT_NUM) // NT != ((t + 1) * ACT_NUM) // NT


def build_program():
    nc = bacc.Bacc("TRN2", target_bir_lowering=False, debug=False)
    xT_d = nc.dram_tensor("xT", [IN_F, N], BF16, kind="ExternalInput").ap()
    mask_d = nc.dram_tensor("mask", [N, N], BF16, kind="ExternalInput").ap()
    W_d = nc.dram_tensor("W", [IN_F, HF], BF16, kind="ExternalInput").ap()
    Wr_d = nc.dram_tensor("Wr", [IN_F, HF], BF16, kind="ExternalInput").ap()
    wi_d = nc.dram_tensor("wi", [HF, 1], BF16, kind="ExternalInput").ap()
    wj_d = nc.dram_tensor("wj", [HF, 1], BF16, kind="ExternalInput").ap()
    eye_d = nc.dram_tensor("eye", [128, 128], BF16, kind="ExternalInput").ap()
    y_d = nc.dram_tensor("y", [N, HF], F32, kind="ExternalOutput").ap()

    with tile.TileContext(nc) as tc, ExitStack() as ctx:
        persist = ctx.enter_context(tc.tile_pool(name="persist", bufs=1))
        h_sb = persist.tile([128, N], BF16, tag="h")  # h[j,f], slice jt -> j-tile
        resid_sb = persist.tile([128, N], BF16, tag="resid")  # resid[i,f] per i-tile
        ciB = persist.tile([128, N], BF16, tag="ciB")  # ci broadcast along partitions
        E1iB = persist.tile([128, N], BF16, tag="E1iB")  # exp(ci) broadcast
        E2iB = persist.tile([128, N], BF16, tag="E2iB")  # exp(0.2 ci) broadcast
        cjT = persist.tile([128, 2 * NJT], F32, tag="cjT")  # cj[j] cols (even idx)
        E1jT = persist.tile([128, 2 * NJT], F32, tag="E1jT")  # exp(cj)
        E2jT = persist.tile([128, 2 * NJT], F32, tag="E2jT")  # exp(0.2 cj)
        eye_bf = persist.tile([128, 128], BF16, tag="eye_bf")
        eye1_sb = persist.tile([64, 1], F32, tag="eye1")
        ones_b = persist.tile([128, 1], BF16, tag="ones")

        nc.sync.dma_start(eye_bf[:], eye_d)
        nc.vector.memset(eye1_sb[:], 1.0)
        nc.vector.memset(ones_b[:], 1.0)

        # Phase-2 pools opened FIRST: their SBUF is disjoint from phase-1
        # buffers, so attention tiles never wait on projection-buffer releases.
        ph2 = ctx.enter_context(tc.tile_pool(name="ph2", bufs=3))
        inpool = ctx.enter_context(tc.tile_pool(name="inpool", bufs=3))
        epool = ctx.enter_context(tc.tile_pool(name="epool", bufs=3))
        fin = ctx.enter_context(tc.tile_pool(name="fin", bufs=2))
        outp = ctx.enter_context(tc.tile_pool(name="outp", bufs=2))

        QC = 1024
        NQ = N // QC  # 4 quarters

        # ---------- helpers ----------

        _etiles = {}

        def emit_elementwise(i0, jt, half, part):
            # part: None = full 2048-wide tile; 0/1 = 1024-wide halves (used
            # by the first j-tiles so part 0 can start after phase-1 quarter
            # 0, overlapping quarter 1)
            t = half * NJT + jt
            if part in (None, 0):
                g_t = ph2.tile([128, HALF], BF16, tag="m", bufs=10, name=f"g_{i0}_{jt}")
                nc.sync.dma_start(
                    g_t[:], mask_d[jt * 128 : (jt + 1) * 128, i0 : i0 + HALF]
                )
                e_t = epool.tile([128, HALF], BF16, tag="e", bufs=6, name=f"e_{i0}_{jt}")
                _etiles[(half, jt)] = (g_t, e_t)
            else:
                g_t, e_t = _etiles[(half, jt)]
            lo = 0 if part in (None, 0) else QC
            w_ = HALF if part is None else QC
            gs = g_t[:, lo : lo + w_]
            es = e_t[:, lo : lo + w_]
            o = i0 + lo
            col = (jt // (NJT // 2)) * NJT + 2 * (jt % (NJT // 2))
            sfx = f"{i0}_{jt}_{part}"
            if _is_act_tile(t):
                u_t = inpool.tile([128, w_], BF16, tag="u", bufs=4, name=f"u_{sfx}")
                nc.scalar.activation(
                    u_t[:],
                    ciB[:, o : o + w_],
                    AF.Prelu,
                    bias=cjT[:, col : col + 1],
                    alpha=SLOPE,
                )
                v_t = inpool.tile([128, w_], BF16, tag="v", bufs=4, name=f"v_{sfx}")
                nc.scalar.activation(v_t[:], u_t[:], AF.Exp)
                nc.vector.tensor_mul(es, v_t[:], gs)
            else:
                u_t = inpool.tile([128, w_], BF16, tag="u", bufs=4, name=f"u_{sfx}")
                nc.vector.tensor_scalar_mul(
                    u_t[:], E1iB[:, o : o + w_], E1jT[:, col : col + 1]
                )
                w_t = inpool.tile([128, w_], BF16, tag="w", bufs=4, name=f"w_{sfx}")
                nc.vector.tensor_scalar_mul(
                    w_t[:], E2iB[:, o : o + w_], E2jT[:, col : col + 1]
                )
                p_t = inpool.tile([128, w_], BF16, tag="v", bufs=4, name=f"p_{sfx}")
                nc.vector.tensor_max(p_t[:], u_t[:], w_t[:])
                nc.vector.tensor_mul(es, p_t[:], gs)
            return e_t

        def emit_jtile(jt, half, yT_ps, rs_ab, part=None):
            i0 = half * HALF
            e_t = emit_elementwise(i0, jt, half, part)
            hr = h_sb[:, jt * 128 : (jt + 1) * 128]
            chunks = range(HALF // 512) if part is None else (
                range(2) if part == 0 else range(2, 4)
            )
            for c in chunks:
                nc.tensor.matmul(
                    yT_ps[:, c * 512 : (c + 1) * 512],
                    hr,
                    e_t[:, c * 512 : (c + 1) * 512],
                    start=(jt == 0),
                    stop=(jt == NJT - 1),
                )
            for c in chunks:
                # rowsum chunks land on partitions 0/32 of two PSUM banks.
                # The banks are pre-zeroed by memset, so every matmul
                # accumulates (start=False); two per-bank interleaved
                # start..stop windows would be UB.
                rs_ps = rs_ab[c // 2]
                p0 = 32 * (c % 2)
                nc.tensor.matmul(
                    rs_ps[p0 : p0 + 1, :],
                    ones_b[:],
                    e_t[:, c * 512 : (c + 1) * 512],
                    start=False,
                    stop=(jt == NJT - 1),
                    skip_group_check=True,
                )

        def emit_finale(half, yT_ps, rs_ab, pfin):
            # rowsum -> reciprocal chain (small, emitted first)
            rs_sb = fin.tile([64, 1024], F32, tag="rs_sb")
            nc.vector.tensor_copy(rs_sb[:, 0:512], rs_ab[0][:])
            nc.vector.tensor_copy(rs_sb[:, 512:1024], rs_ab[1][:])
            rsT_ps = pfin.tile([128, HALF // 128], F32, tag="rsT")
            for b in range(HALF // 128):
                c, off = b // 4, (b % 4) * 128
                p0 = 32 * (c % 2)
                nc.tensor.transpose(
                    rsT_ps[:, b : b + 1],
                    rs_sb[p0 : p0 + 1, (c // 2) * 512 + off : (c // 2) * 512 + off + 128],
                    eye1_sb[p0 : p0 + 1, 0:1],
                )
            rsT_sb = fin.tile([128, HALF // 128], F32, tag="rsT_sb")
            nc.vector.tensor_copy(rsT_sb[:], rsT_ps[:])
            recipT = fin.tile([128, HALF // 128], F32, tag="recipT")
            nc.vector.reciprocal(recipT[:], rsT_sb[:])

            # yT strips: copy -> transpose -> combine -> DMA, pipelined
            ytr_sb = fin.tile([128, HALF], BF16, tag="ytr_sb")
            for s in range(HALF // 512):
                yT_sb = fin.tile([128, 512], BF16, tag="yT_sb", bufs=3, name=f"yts_{half}_{s}")
                nc.vector.tensor_copy(yT_sb[:], yT_ps[:, s * 512 : (s + 1) * 512])
                tr_ps = pfin.tile([128, 512], BF16, tag="tr", bufs=1, name=f"tr_{half}_{s}")
                for k in range(4):
                    nc.tensor.transpose(
                        tr_ps[:, k * 128 : (k + 1) * 128],
                        yT_sb[:, k * 128 : (k + 1) * 128],
                        eye_bf[:],
                    )
                nc.vector.tensor_copy(
                    ytr_sb[:, s * 512 : (s + 1) * 512], tr_ps[:]
                )
            return ytr_sb, recipT

        def emit_combine(half, ytr_sb, recipT):
            for gi in range(HALF // 128):
                g = half * (HALF // 128) + gi
                ob = outp.tile([128, HF], F32, tag="ob", bufs=3)
                nc.vector.scalar_tensor_tensor(
                    ob[:],
                    ytr_sb[:, gi * 128 : (gi + 1) * 128],
                    recipT[:, gi : gi + 1],
                    resid_sb[:, g * 128 : (g + 1) * 128],
                    op0=OP.mult,
                    op1=OP.add,
                )
                nc.sync.dma_start(y_d[g * 128 : (g + 1) * 128, :], ob[:])

        # ---------- program ----------

        # pmm0 opens BEFORE the phase-1 pools: tile pools close LIFO, and
        # phase-1 (psb, 2 banks) must release while pmm0 (6 banks) stays live.
        pmm0_ctx = ExitStack()
        pmm0 = pmm0_ctx.enter_context(tc.tile_pool(name="pmm0", bufs=1, space="PSUM"))
        yT0 = pmm0.tile([128, HALF], F32, tag="yT", name="yT0")
        rs0 = (
            pmm0.tile([64, 512], F32, tag="rsa", name="rs0a"),
            pmm0.tile([64, 512], F32, tag="rsb", name="rs0b"),
        )
        nc.vector.memset(rs0[0][:], 0.0)
        nc.vector.memset(rs0[1][:], 0.0)

        with ExitStack() as p1:
            ph1 = p1.enter_context(tc.tile_pool(name="ph1", bufs=1))
            xpool = p1.enter_context(tc.tile_pool(name="xpool", bufs=3))
            psb = p1.enter_context(tc.tile_pool(name="psb", bufs=1, space="PSUM"))

            W_sb = ph1.tile([128, NMC * HF], BF16, tag="W")
            Wr_sb = ph1.tile([128, NMC * HF], BF16, tag="Wr")
            # one strided DMA each: dst col block mc <- DRAM rows mc*128..
            nc.sync.dma_start(
                W_sb[:], W_d.rearrange("(m p) f -> p m f", m=NMC)
            )
            nc.sync.dma_start(
                Wr_sb[:], Wr_d.rearrange("(m p) f -> p m f", m=NMC)
            )
            wi_sb = ph1.tile([128, 1], BF16, tag="wi")
            nc.sync.dma_start(wi_sb[:], wi_d)
            wj_sb = ph1.tile([128, 1], BF16, tag="wj")
            nc.sync.dma_start(wj_sb[:], wj_d)
            # wj padded to 2 columns so the moving free dim stays 4B-aligned
            wj2 = ph1.tile([128, 2], BF16, tag="wj2")
            nc.vector.memset(wj2[:], 0.0)
            nc.vector.tensor_copy(wj2[:, 0:1], wj_sb[:])

            hT_sb = ph1.tile([128, N], BF16, tag="hT")  # hT[f, j]
            ci_rowh = ph1.tile([1, N], BF16, tag="ci_row")

            # phase-1 PSUM: psA 1 bank + psC 1 bank = 2 banks, so it coexists
            # with the half-0 attention accumulators (4 yT + 2 rs banks).
            # hT accumulates in psA while resid accumulates in psC (separate
            # windows, pipelined per-mc); ci reuses psA after the hT evac.
            xq_tiles = {}

            def p_xload(q):
                # quarter-wide x loads: 4 DMAs of [128, 1024] per quarter
                o = q * QC
                xts = []
                for mc in range(NMC):
                    xt = xpool.tile(
                        [128, QC], BF16, tag="xt", bufs=6, name=f"xt_{q}_{mc}"
                    )
                    nc.sync.dma_start(
                        xt[:], xT_d[mc * 128 : (mc + 1) * 128, o : o + QC]
                    )
                    xts.append(xt)
                xq_tiles[q] = xts

            def p_subchunk(q, sc):
                oo = q * QC + sc * 512
                ps_hT = psb.tile([128, 512], F32, tag="psA", name=f"phT_{q}_{sc}")
                ps_res = psb.tile([128, 512], F32, tag="psC", name=f"pres_{q}_{sc}")
                for mc in range(NMC):
                    xt = xq_tiles[q][mc][:, sc * 512 : (sc + 1) * 512]
                    nc.tensor.matmul(
                        ps_hT[:],
                        W_sb[:, mc * HF : (mc + 1) * HF],
                        xt,
                        start=(mc == 0),
                        stop=(mc == NMC - 1),
                    )
                    for it in range(4):
                        # exactly ONE start per PSUM bank (first write opens
                        # the accumulation window; overlapping windows are UB)
                        nc.tensor.matmul(
                            ps_res[:, it * 128 : (it + 1) * 128],
                            xt[:, it * 128 : (it + 1) * 128],
                            Wr_sb[:, mc * HF : (mc + 1) * HF],
                            start=(mc == 0 and it == 0),
                            stop=(mc == NMC - 1),
                        )
                nc.vector.tensor_copy(hT_sb[:, oo : oo + 512], ps_hT[:])
                # ci for this subchunk reuses psA once the hT evac clears it
                ps_c = psb.tile([128, 512], F32, tag="psA", name=f"ci_{q}_{sc}")
                nc.tensor.matmul(
                    ps_c[0:1, :],
                    wi_sb[:],
                    hT_sb[:, oo : oo + 512],
                    start=True,
                    stop=True,
                )
                nc.vector.tensor_copy(ci_rowh[0:1, oo : oo + 512], ps_c[0:1, :])
                nc.vector.tensor_copy(resid_sb[:, oo : oo + 512], ps_res[:])

            def p_rows(q):
                o = q * QC
                JQ = QC // 128
                # cj columns for this quarter of j-tiles (8 per quarter).
                # psA is reused here (after the res evac) so cj's accumulation
                # never lands on a bank holding live ci rows.
                ps_c = psb.tile([128, 512], F32, tag="psA", name=f"cj_{q}")
                for k in range(JQ):
                    jt = q * JQ + k
                    nc.tensor.matmul(
                        ps_c[:, 2 * k : 2 * k + 2],
                        hT_sb[:, jt * 128 : (jt + 1) * 128],
                        wj2[:],
                        start=(k == 0),
                        stop=(k == JQ - 1),
                    )
                co = q * 2 * JQ
                nc.vector.tensor_copy(cjT[:, co : co + 2 * JQ], ps_c[:, 0 : 2 * JQ])
                nc.scalar.activation(E1jT[:, co : co + 2 * JQ], ps_c[:, 0 : 2 * JQ], AF.Exp)
                nc.scalar.activation(
                    E2jT[:, co : co + 2 * JQ], ps_c[:, 0 : 2 * JQ], AF.Exp, scale=SLOPE
                )
                nc.gpsimd.partition_broadcast(ciB[:, o : o + QC], ci_rowh[0:1, o : o + QC])
                nc.scalar.activation(E1iB[:, o : o + QC], ciB[:, o : o + QC], AF.Exp)
                nc.scalar.activation(
                    E2iB[:, o : o + QC], ciB[:, o : o + QC], AF.Exp, scale=SLOPE
                )

            def p_htr(q, sc):
                # h[j, f] for 4 j-tiles = transpose(hT) blockwise
                oo = q * QC + sc * 512
                ps_h = psb.tile([128, 512], BF16, tag="psA", name=f"ph_{q}_{sc}")
                for k in range(4):
                    jt = (oo // 128) + k
                    nc.tensor.transpose(
                        ps_h[:, k * 128 : (k + 1) * 128],
                        hT_sb[:, jt * 128 : (jt + 1) * 128],
                        eye_bf[:],
                    )
                nc.vector.tensor_copy(h_sb[:, oo : oo + 512], ps_h[:])

            def emit_quarter(q):
                p_xload(q)
                p_subchunk(q, 0)
                p_subchunk(q, 1)
                p_rows(q)
                p_htr(q, 0)
                p_htr(q, 1)

            # phase-1 quarters 0,1 give half-0 everything it needs for
            # j-tiles 0..15; quarters 2,3 emit piecewise between those tiles
            # so their projections overlap the attention steady state without
            # head-of-line blocking the per-tile e pipeline.
            emit_quarter(0)
            # part-0 of the first 6 j-tiles only needs quarter-0 data;
            # their e-gen overlaps quarter 1's projections
            for jt in range(6):
                emit_jtile(jt, 0, yT0, rs0, part=0)
            emit_quarter(1)
            for jt in range(6):
                emit_jtile(jt, 0, yT0, rs0, part=1)
            for jt in range(6, 8):
                emit_jtile(jt, 0, yT0, rs0)
            q2 = [
                lambda: (p_xload(2), p_subchunk(2, 0)),
                lambda: p_subchunk(2, 1),
                lambda: p_rows(2),
                lambda: p_htr(2, 0),
                lambda: p_htr(2, 1),
            ]
            for jt in range(8, 13):
                emit_jtile(jt, 0, yT0, rs0)
                q2[jt - 8]()
            for jt in range(13, 16):
                emit_jtile(jt, 0, yT0, rs0)
            q3 = [
                lambda: (p_xload(3), p_subchunk(3, 0)),
                lambda: p_subchunk(3, 1),
                lambda: p_rows(3),
                lambda: p_htr(3, 0),
                lambda: p_htr(3, 1),
            ]
            for jt in range(16, 21):
                emit_jtile(jt, 0, yT0, rs0)
                q3[jt - 16]()
            for jt in range(21, NJT):
                emit_jtile(jt, 0, yT0, rs0)

        # half-0 finale + half-1 loop
        with ExitStack() as pf_ctx:
            pfin0 = pf_ctx.enter_context(
                tc.tile_pool(name="pfin0", bufs=1, space="PSUM")
            )
            ytr0, recip0 = emit_finale(0, yT0, rs0, pfin0)
        pmm0_ctx.close()
        emit_combine(0, ytr0, recip0)

        with ExitStack() as pmm1_ctx:
            pmm1 = pmm1_ctx.enter_context(
                tc.tile_pool(name="pmm1", bufs=1, space="PSUM")
            )
            yT1 = pmm1.tile([128, HALF], F32, tag="yT")
            rs1 = (
                pmm1.tile([64, 512], F32, tag="rsa", name="rs1a"),
                pmm1.tile([64, 512], F32, tag="rsb", name="rs1b"),
            )
            nc.vector.memset(rs1[0][:], 0.0)
            nc.vector.memset(rs1[1][:], 0.0)
            for jt in range(NJT):
                emit_jtile(jt, 1, yT1, rs1)

            with ExitStack() as pf_ctx:
                pfin1 = pf_ctx.enter_context(
                    tc.tile_pool(name="pfin1", bufs=1, space="PSUM")
                )
                ytr1, recip1 = emit_finale(1, yT1, rs1, pfin1)
        emit_combine(1, ytr1, recip1)

    nc.compile()
    return nc


def _get_program():
    global _prog
    if _prog is None:
        _prog = build_program()
    return _prog


def _prepare_in_maps(x, graph, W, w_i, w_j, W_r):
    bf = ml_dtypes.bfloat16
    xT = np.ascontiguousarray(x.T).astype(bf)
    mask = (graph > 0).astype(bf)  # multiplicative 0/1 mask
    eye = np.eye(128, dtype=ml_dtypes.bfloat16)
    in_maps = []
    for c in range(HEADS):
        in_maps.append(
            {
                "xT": xT,
                "mask": mask,
                "W": np.ascontiguousarray(W[c]).astype(bf),
                "Wr": np.ascontiguousarray(W_r[:, c * HF : (c + 1) * HF]).astype(bf),
                "wi": np.ascontiguousarray(w_i[c]).astype(bf),
                "wj": np.ascontiguousarray(w_j[c]).astype(bf),
                "eye": eye,
            }
        )
    return in_maps


def run(inputs, trace=False, **kwargs):
    """Run the SPMD kernel; returns (y_full, BassKernelResults)."""
    x = np.asarray(inputs["x"], dtype=np.float32)
    graph = np.asarray(inputs["graph"])
    W = np.asarray(inputs["W"], dtype=np.float32)
    w_i = np.asarray(inputs["w_i"], dtype=np.float32)
    w_j = np.asarray(inputs["w_j"], dtype=np.float32)
    W_r = np.asarray(inputs["W_r"], dtype=np.float32)
    bias = np.asarray(inputs["bias"], dtype=np.float32)

    nc = _get_program()
    in_maps = _prepare_in_maps(x, graph, W, w_i, w_j, W_r)
    br = run_bass_kernel_spmd(
        nc, in_maps, core_ids=list(range(HEADS)), trace=trace, **kwargs
    )
    y = np.concatenate([br.results[c]["y"] for c in range(HEADS)], axis=1)
    y = y + bias[None, :]
    return y.astype(np.float32), br


def kernel(**inputs):
    y, _ = run(inputs)
    return y
